# revision 39
# baseline (speedup 1.0000x reference)
"""BEiT-style windowed attention (B=128, N=197, C=768, H=12) on 8 TRN2 NeuronCores.

Data-parallel over batch: 16 batches per core, 2-batch half-blocks inside
4-batch superblocks. Host pre-processing casts x and the qkv/v/proj weights to
bf16, folds the attention scale into the q weights/bias, folds v_bias into the
projection bias (softmax rows sum to 1), and pre-gathers exp(rel_pos_bias).

Device pipeline per core, per 2-batch half-block:
  qkT  [1536, 394] = qk_wT.T @ xT      (bf16 matmuls, moving dim 394)
  v    [394, 768]  = xT.T @ v_wT       (bf16) with interleaved ones columns
  S.T  [197, 197]  = kT.T @ qT         (bf16 per head; both batches land in one
                                        [128,394] psum as two closed groups)
  E    = exp(S.T) * exp_rb             (one ACT exp per psum — ACT ops have
                                        ~530ns fixed overhead, so fewer+wider
                                        wins; exp(rb) multiply split DVE/Pool;
                                        no max-subtraction: |scores| < ~3)
  outT [128, 197]  = v_aug.T @ E       (cols 64:128 of v_aug are ones -> rows
                                        64:128 of outT are the softmax sums;
                                        both heads of a pair share one psum)
  attnoutT = outT[0:64] * recip(outT[64:128])   (reciprocal on ACT — the DVE
                                        reciprocal measures 3.2us/op on HW)
  out  = attnoutT.T @ proj_wT + bias   (bf16, projected once per 4-batch
                                        superblock: 7 M-tiles instead of 8;
                                        bias added via a pre-broadcast tensor)
"""
import sys
sys.path.insert(0, '/opt/trn_rl_repo')

import numpy as np
import ml_dtypes
from contextlib import ExitStack

import concourse.bass as bass
import concourse.tile as tile
from concourse.tile import add_dep_helper
from concourse import mybir
from concourse.bass_utils import run_bass_kernel_spmd
from concourse.vector_clock import ScopedClock, VectorClock

f32 = mybir.dt.float32
f32r = mybir.dt.float32r
bf16 = mybir.dt.bfloat16

N_CORES = 8
RB_MODE = "ident_pe"
B, N, C, H, HD = 128, 197, 768, 12, 64
BC = B // N_CORES          # batches per core
BLK = 2                    # batches per block
NB = BC // BLK             # blocks per core
NP = BLK * N               # block column width (394)
SCALE = HD ** -0.5


class TileContextFixed(tile.TileContext):
    """The walrus in this container accepts at most ONE sync wait per
    instruction. Stock Tile attaches several (both on ordinary instructions
    during wait assignment and on the tail drain). Split the extras onto
    same-engine InstNoOps, and emit the tail drain one proc at a time."""

    def _lower_ordered_insts(self, ordered):
        for bb_name, insts in ordered.items():
            i = 0
            while i < len(insts):
                inst = insts[i]
                si = inst.sync_info
                if si is not None and si.on_wait and len(si.on_wait) > 1:
                    waits = list(si.on_wait)
                    inst.sync_info = mybir.SyncInfo(
                        on_wait=[waits[-1]], on_update=list(si.on_update)
                    )
                    nops = [
                        mybir.InstNoOp(
                            name=f"{inst.name}__wsplit{k}",
                            engine=inst.engine,
                            bass_nofuse=True,
                            sync_info=mybir.SyncInfo(on_wait=[w], on_update=[]),
                        )
                        for k, w in enumerate(waits[:-1])
                    ]
                    insts[i:i] = nops
                    i += len(nops)
                i += 1
        return super()._lower_ordered_insts(ordered)

    def _drain_and_barrier(self, tick_clock, wait_clock):
        gc = tick_clock.global_clock
        n = len(gc)
        for i in range(n):
            if gc[i] > 0:
                vc = VectorClock([0] * n)
                vc.require_at_least(i, gc[i])
                d = self.nc.sync.drain()
                wait_clock.add_sem_waits(d.ins, ScopedClock({None: vc}))
        self.nc.all_engine_barrier()
        assert self.sems is not None
        popped = self.nc._tile_sem_poison_stack.pop()
        assert popped is self._sem_poison
        self.nc.clear_and_free_semaphores(list(self.sems.allocated().values()))
        self.nc.all_engine_barrier()


def _act_recip(eng, out, in_):
    imm = lambda v: mybir.ImmediateValue(dtype=f32, value=v)
    return eng.add_instruction(mybir.InstActivation(
        name=eng.bass.get_next_instruction_name(),
        func=mybir.ActivationFunctionType.Reciprocal,
        ins=[eng.lower_ap(in_), imm(0.0), imm(1.0), imm(0.0)],
        outs=[eng.lower_ap(out)],
    ))


def build_nc(rb_mode=RB_MODE, patt_bufs=3, pmm_bufs=3, ppv_bufs=2, e_bufs=10):
    # rb_mode: how exp(S+rb) is formed:
    #   "mul_pool"  E = exp(S) * erb on gpsimd
    #   "mul_dve"   E = exp(S) * erb on DVE
    #   "mul_split" alternate gpsimd/DVE by head parity
    #   "ident_pe"  S += rb via identity matmul on PE, E = exp(S)
    nc = bass.Bass("TRN2", target_bir_lowering=False, debug=False)
    Exp = mybir.ActivationFunctionType.Exp

    xT_d = nc.dram_tensor("xT", [BC, C, N], bf16, kind="ExternalInput").ap()
    qkw_d = nc.dram_tensor("qkw", [C, 2 * C], bf16, kind="ExternalInput").ap()
    vw_d = nc.dram_tensor("vw", [C, C], bf16, kind="ExternalInput").ap()
    pw_d = nc.dram_tensor("pw", [C, C], bf16, kind="ExternalInput").ap()
    pb_d = nc.dram_tensor("pb", [1, C], f32, kind="ExternalInput").ap()
    qb_d = nc.dram_tensor("qb", [128, 6], f32, kind="ExternalInput").ap()

    erb_d = nc.dram_tensor("erb", [H, N, NP], bf16, kind="ExternalInput").ap()
    out_d = nc.dram_tensor("out", [BC * N, C], f32, kind="ExternalOutput").ap()

    MT = ((0, 128), (128, 69))  # (row offset, rows) m-tiles of 197

    with TileContextFixed(nc) as tc, ExitStack() as ctx:
        consts = ctx.enter_context(tc.tile_pool(name="consts", bufs=1))
        xt_p = ctx.enter_context(tc.tile_pool(name="xt", bufs=2))
        qkt_p = ctx.enter_context(tc.tile_pool(name="qkt", bufs=3))
        v_p = ctx.enter_context(tc.tile_pool(name="v", bufs=2))
        at_p = ctx.enter_context(tc.tile_pool(name="at", bufs=3))
        e_p = ctx.enter_context(tc.tile_pool(name="e", bufs=e_bufs))
        rcp_p = ctx.enter_context(tc.tile_pool(name="rcp", bufs=4))
        stage_p = ctx.enter_context(tc.tile_pool(name="stage", bufs=3))
        pmm = ctx.enter_context(tc.tile_pool(name="pmm", bufs=pmm_bufs, space="PSUM"))
        patt = ctx.enter_context(tc.tile_pool(name="patt", bufs=patt_bufs, space="PSUM"))
        ppv = ctx.enter_context(tc.tile_pool(name="ppv", bufs=ppv_bufs, space="PSUM"))

        xt_pre = xt_p.tile([128, 6, NP], bf16)
        for j in range(BLK):
            nc.sync.dma_start(
                out=xt_pre[:, :, j * N:(j + 1) * N],
                in_=xT_d[j].rearrange("(k p) n -> p k n", p=128),
            )
        qkw_s = consts.tile([128, 6, 2 * C], bf16)
        qkw_r = qkw_d.rearrange("(k p) c -> p k c", p=128)
        for k in range(6):
            nc.sync.dma_start(out=qkw_s[:, k, :], in_=qkw_r[:, k, :])
        qb_s = consts.tile([128, 6], f32)
        nc.sync.dma_start(out=qb_s[:], in_=qb_d[:])
        vw_s = consts.tile([128, 6, C], bf16)
        pw_s = consts.tile([128, 6, C], bf16)
        erb0_s = consts.tile([128, H, NP], bf16)
        erb1_s = consts.tile([69, H, NP], bf16)
        pbb_s = consts.tile([128, C], f32)

        SB = NB // 2                      # superblocks of 4 batches
        MT7 = [(g, min(128, 2 * NP - g)) for g in range(0, 2 * NP, 128)]

        for sb in range(SB):
            at_s = at_p.tile([128, 6, 2 * NP], bf16)
            for bh in range(2):
                blk = sb * 2 + bh
                b0 = blk * BLK
                off = bh * NP

                if blk == 0:
                    xt_s = xt_pre
                else:
                    xt_s = xt_p.tile([128, 6, NP], bf16)
                    for j in range(BLK):
                        nc.sync.dma_start(
                            out=xt_s[:, :, j * N:(j + 1) * N],
                            in_=xT_d[b0 + j].rearrange("(k p) n -> p k n", p=128),
                        )

                # ---- qkT [12 x 128, NP] bf16 (q part gets scaled bias) ----
                qkt_s = qkt_p.tile([128, H, NP], bf16)
                anchors = {}
                for mi in range(12):
                    ps = pmm.tile([128, NP], f32, tag="pmm")
                    for k in range(6):
                        mm = nc.tensor.matmul(
                            ps[:],
                            lhsT=qkw_s[:, k, mi * 128:(mi + 1) * 128],
                            rhs=xt_s[:, k, :],
                            start=(k == 0), stop=(k == 5),
                        )
                        if blk == 0 and mi in (0, 6) and k == 0:
                            anchors[mi] = mm.ins
                    if mi < 6:
                        nc.vector.tensor_scalar_add(qkt_s[:, mi, :], ps[:], qb_s[:, mi:mi + 1])
                    else:
                        nc.vector.tensor_copy(out=qkt_s[:, mi, :], in_=ps[:])

                if blk == 0:
                    d1 = nc.gpsimd.dma_start(out=vw_s[:], in_=vw_d.rearrange("(k p) c -> p k c", p=128))
                    d2 = nc.gpsimd.dma_start(out=erb0_s[:], in_=erb_d[:, 0:128, :].rearrange("h p n -> p h n"))
                    d3 = nc.gpsimd.dma_start(out=erb1_s[:], in_=erb_d[:, 128:197, :].rearrange("h p n -> p h n"))
                    d4 = nc.gpsimd.dma_start(out=pw_s[:], in_=pw_d.rearrange("(k p) c -> p k c", p=128))
                    d5 = nc.gpsimd.dma_start(out=pbb_s[:], in_=bass.AP(tensor=pb_d.tensor, offset=0,
                                                                       ap=[[0, 128], [1, C]]))
                    for d in (d1, d2, d3):
                        add_dep_helper(d.ins, anchors[0], reason="defer const load past startup")
                    for d in (d4, d5):
                        add_dep_helper(d.ins, anchors[6], reason="defer const load past startup")

                # ---- v natural [NP, 12*(64 v | 64 ones)] bf16 ----
                v_s = v_p.tile([128, BLK, 2, H, 128], bf16)
                nc.gpsimd.memset(v_s[:, :, :, :, 64:128], 1.0)
                for j in range(BLK):
                    for t, (r0, msz) in enumerate(MT):
                        for nt in range(2):
                            ps = pmm.tile([128, 384], f32, tag="pmm")
                            for k in range(6):
                                nc.tensor.matmul(
                                    ps[0:msz, :],
                                    lhsT=xt_s[:, k, j * N + r0: j * N + r0 + msz],
                                    rhs=vw_s[:, k, nt * 384:(nt + 1) * 384],
                                    start=(k == 0), stop=(k == 5),
                                )
                            nc.vector.tensor_copy(
                                out=v_s[0:msz, j, t, nt * 6:(nt + 1) * 6, 0:64],
                                in_=ps[0:msz, :].rearrange("p (h d) -> p h d", h=6),
                            )

                # ---- attention: scores psum holds both batches of the half-block
                # as two CLOSED groups; one exp + one exp(rb)-multiply ----
                for hp in range(6):
                    es = {}
                    for t, (r0, msz) in enumerate(MT):
                        erb_t = erb0_s if t == 0 else erb1_s
                        # Interleave the two heads' K=64 scores matmuls so each
                        # adjacent PE instruction targets a disjoint row group
                        # (0:64 vs 64:128) and the sub-arrays overlap them.
                        ps_a = patt.tile([128, NP], f32, tag="patt")
                        ps_b = patt.tile([128, NP], f32, tag="patt")
                        pss = {0: ps_a, 1: ps_b}
                        for j in range(BLK):
                            for hi in range(2):
                                nc.tensor.matmul(
                                    pss[hi][0:msz, j * N:(j + 1) * N],
                                    lhsT=qkt_s[64 * hi:64 * (hi + 1), 6 + hp,
                                               j * N + r0: j * N + r0 + msz],
                                    rhs=qkt_s[64 * hi:64 * (hi + 1), hp, j * N:(j + 1) * N],
                                    start=True, stop=True, skip_group_check=True,
                                )
                        for hi in range(2):
                            h = 2 * hp + hi
                            e = e_p.tile([128, NP], bf16, tag="e")
                            nc.scalar.activation(out=e[0:msz, :], in_=pss[hi][0:msz, :], func=Exp)
                            eng = nc.gpsimd if hi == 0 else nc.vector
                            eng.tensor_mul(e[0:msz, :], e[0:msz, :], erb_t[0:msz, h, :])
                            es[(t, hi)] = e
                    for j in range(BLK):
                        ps_o = ppv.tile([128, 2 * N], f32, tag="ppv")
                        for hi in range(2):
                            h = 2 * hp + hi
                            for t, (r0, msz) in enumerate(MT):
                                nc.tensor.matmul(
                                    ps_o[:, hi * N:(hi + 1) * N],
                                    lhsT=v_s[0:msz, j, t, h, :],
                                    rhs=es[(t, hi)][0:msz, j * N:(j + 1) * N],
                                    start=(t == 0), stop=(t == 1),
                                )
                        rcp = rcp_p.tile([64, 2 * N], f32, tag="rcp")
                        _act_recip(nc.scalar, rcp[:], ps_o[64:128, :])
                        for hi in range(2):
                            nc.vector.tensor_mul(
                                at_s[hi * 64:hi * 64 + 64, hp, off + j * N:off + (j + 1) * N],
                                ps_o[0:64, hi * N:(hi + 1) * N], rcp[:, hi * N:(hi + 1) * N],
                            )

            # ---- projection over the whole superblock (flat rows), + bias ----
            for g0, msz in MT7:
                stage = stage_p.tile([128, C], f32)
                for nt in range(2):
                    ps = pmm.tile([128, 384], f32, tag="pmm")
                    for k in range(6):
                        nc.tensor.matmul(
                            ps[0:msz, :],
                            lhsT=at_s[:, k, g0:g0 + msz],
                            rhs=pw_s[:, k, nt * 384:(nt + 1) * 384],
                            start=(k == 0), stop=(k == 5),
                        )
                    nc.vector.scalar_tensor_tensor(
                        out=stage[0:msz, nt * 384:(nt + 1) * 384],
                        in0=ps[0:msz, :], scalar=1.0,
                        in1=pbb_s[0:msz, nt * 384:(nt + 1) * 384],
                        op0=mybir.AluOpType.mult, op1=mybir.AluOpType.add,
                    )
                nc.sync.dma_start(
                    out=out_d[sb * 2 * NP + g0: sb * 2 * NP + g0 + msz, :],
                    in_=stage[0:msz, :],
                )
    return nc


_NC = None


def _get_nc():
    global _NC
    if _NC is None:
        _NC = build_nc()
    return _NC


_EXEC = None


def _get_exec():
    """Build the sharded PJRT executable once and reuse it across calls
    (run_bass_via_pjrt re-traces jax.jit on every invocation)."""
    global _EXEC
    if _EXEC is not None:
        return _EXEC
    import jax
    import numpy as _np
    from jax.sharding import Mesh, PartitionSpec
    from jax.experimental.shard_map import shard_map
    import concourse.mybir as mybir_
    from concourse import bass2jax

    nc = _get_nc()
    bass2jax.install_neuronx_cc_hook()
    partition_name = nc.partition_id_tensor.name if nc.partition_id_tensor else None
    in_names, out_names, out_avals = [], [], []
    for alloc in nc.m.functions[0].allocations:
        if not isinstance(alloc, mybir_.MemoryLocationSet):
            continue
        name = alloc.memorylocations[0].name
        if alloc.kind == "ExternalInput":
            if name != partition_name:
                in_names.append(name)
        elif alloc.kind == "ExternalOutput":
            out_names.append(name)
            out_avals.append(jax.core.ShapedArray(
                tuple(alloc.tensor_shape), mybir_.dt.np(alloc.dtype)))
    all_names = list(in_names)
    if partition_name is not None:
        all_names = all_names + [partition_name]

    def _body(*args):
        operands = list(args)
        if partition_name is not None:
            operands.append(bass2jax.partition_id_tensor())
        outs = bass2jax._bass_exec_p.bind(
            *operands,
            out_avals=tuple(out_avals),
            in_names=tuple(all_names),
            out_names=tuple(out_names),
            lowering_input_output_aliases=(),
            sim_require_finite=True,
            sim_require_nnan=True,
            nc=nc,
        )
        return tuple(outs)

    devices = jax.devices()[:N_CORES]
    mesh = Mesh(_np.asarray(devices), ("core",))
    # xT is data-parallel (split on axis 0); every other input is replicated,
    # so it uploads once instead of 8x.
    in_specs = tuple(
        PartitionSpec("core") if name == "xT" else PartitionSpec()
        for name in in_names
    )
    out_specs = (PartitionSpec("core"),) * len(out_avals)
    sharded = jax.jit(
        shard_map(_body, mesh=mesh, in_specs=in_specs, out_specs=out_specs,
                  check_rep=False),
        keep_unused=True,
    )
    _EXEC = (sharded, in_names, out_names, out_avals)
    return _EXEC


def _prep_host(x, qkv_w, q_bias, v_bias, rel_pos_table, proj_w, proj_b, rel_index,
               rb_mode="mul_pool"):
    x = np.asarray(x, np.float32)
    qkv_w = np.asarray(qkv_w, np.float32)
    xT = np.ascontiguousarray(x.transpose(0, 2, 1)).astype(ml_dtypes.bfloat16)
    qk_wT = np.ascontiguousarray(qkv_w[:2 * C].T)              # [C, 2C]
    qk_wT[:, :C] *= SCALE
    qk_wT = qk_wT.astype(ml_dtypes.bfloat16)
    qb = (np.asarray(q_bias, np.float32) * SCALE).reshape(6, 128).T.copy()  # [128, 6]
    v_wT = np.ascontiguousarray(qkv_w[2 * C:].T).astype(ml_dtypes.bfloat16)
    proj_wT = np.ascontiguousarray(np.asarray(proj_w, np.float32).T).astype(ml_dtypes.bfloat16)
    pb_eff = (np.asarray(proj_b, np.float32)
              + np.asarray(proj_w, np.float32) @ np.asarray(v_bias, np.float32))
    rb = np.asarray(rel_pos_table, np.float32)[
        np.asarray(rel_index).reshape(-1)].reshape(N, N, H)    # [n, m, h]
    rbT = np.exp(rb.transpose(2, 1, 0))
    rbT = np.concatenate([rbT] * BLK, axis=2)
    erbT = rbT.astype(ml_dtypes.bfloat16)
    return xT, qk_wT, qb, v_wT, proj_wT, pb_eff.reshape(1, C), erbT


def kernel(x, qkv_w, q_bias, v_bias, rel_pos_table, proj_w, proj_b, rel_index):
    xT, qk_wT, qb, v_wT, proj_wT, pb_eff, erbT = _prep_host(
        x, qkv_w, q_bias, v_bias, rel_pos_table, proj_w, proj_b, rel_index,
        rb_mode=RB_MODE)
    per_core = {
        "xT": xT,                                   # [B, C, N] -> split on axis 0
        "qkw": qk_wT, "vw": v_wT, "pw": proj_wT,
        "pb": pb_eff, "qb": qb, "erb": erbT,
    }
    try:
        sharded, in_names, out_names, out_avals = _get_exec()
        concat_in = [np.ascontiguousarray(per_core[name]) for name in in_names]
        out_arrs = sharded(*concat_in)
        out = np.asarray(out_arrs[out_names.index("out")]).reshape(B, N, C)
    except Exception:
        # Robust fallback: the stock SPMD runner (slower per call, same NEFF).
        in_maps = []
        for c in range(N_CORES):
            m = {k: v for k, v in per_core.items() if k != "xT"}
            m["xT"] = np.ascontiguousarray(xT[c * BC:(c + 1) * BC])
            in_maps.append(m)
        res = run_bass_kernel_spmd(_get_nc(), in_maps, core_ids=list(range(N_CORES)))
        out = np.concatenate(
            [res.results[c]["out"].reshape(BC, N, C) for c in range(N_CORES)], axis=0)
    return out.astype(np.float32)


# revision 40
# speedup vs baseline: 1.0155x; 1.0155x over previous
"""BEiT-style windowed attention (B=128, N=197, C=768, H=12) on 8 TRN2 NeuronCores.

Data-parallel over batch: 16 batches per core, 2-batch half-blocks inside
4-batch superblocks. Host pre-processing casts x and the qkv/v/proj weights to
bf16, folds the attention scale into the q weights/bias, folds v_bias into the
projection bias (softmax rows sum to 1), and pre-gathers exp(rel_pos_bias).

Device pipeline per core, per 2-batch half-block:
  qkT  [1536, 394] = qk_wT.T @ xT      (bf16 matmuls, moving dim 394)
  v    [394, 768]  = xT.T @ v_wT       (bf16) with interleaved ones columns
  S.T  [197, 197]  = kT.T @ qT         (bf16 per head; both batches land in one
                                        [128,394] psum as two closed groups)
  E    = exp(S.T) * exp_rb             (one ACT exp per psum — ACT ops have
                                        ~530ns fixed overhead, so fewer+wider
                                        wins; exp(rb) multiply split DVE/Pool;
                                        no max-subtraction: |scores| < ~3)
  outT [128, 197]  = v_aug.T @ E       (cols 64:128 of v_aug are ones -> rows
                                        64:128 of outT are the softmax sums;
                                        both heads of a pair share one psum)
  attnoutT = outT[0:64] * recip(outT[64:128])   (reciprocal on ACT — the DVE
                                        reciprocal measures 3.2us/op on HW)
  out  = attnoutT.T @ proj_wT + bias   (bf16, projected once per 4-batch
                                        superblock: 7 M-tiles instead of 8;
                                        bias added via a pre-broadcast tensor)
"""
import sys
sys.path.insert(0, '/opt/trn_rl_repo')

import numpy as np
import ml_dtypes
from contextlib import ExitStack

import concourse.bass as bass
import concourse.tile as tile
from concourse.tile import add_dep_helper
from concourse import mybir
from concourse.bass_utils import run_bass_kernel_spmd
from concourse.vector_clock import ScopedClock, VectorClock

f32 = mybir.dt.float32
f32r = mybir.dt.float32r
bf16 = mybir.dt.bfloat16

N_CORES = 8
RB_MODE = "ident_pe"
B, N, C, H, HD = 128, 197, 768, 12, 64
BC = B // N_CORES          # batches per core
BLK = 2                    # batches per block
NB = BC // BLK             # blocks per core
NP = BLK * N               # block column width (394)
SCALE = HD ** -0.5


class TileContextFixed(tile.TileContext):
    """The walrus in this container accepts at most ONE sync wait per
    instruction. Stock Tile attaches several (both on ordinary instructions
    during wait assignment and on the tail drain). Split the extras onto
    same-engine InstNoOps, and emit the tail drain one proc at a time."""

    def _lower_ordered_insts(self, ordered):
        for bb_name, insts in ordered.items():
            i = 0
            while i < len(insts):
                inst = insts[i]
                si = inst.sync_info
                if si is not None and si.on_wait and len(si.on_wait) > 1:
                    waits = list(si.on_wait)
                    inst.sync_info = mybir.SyncInfo(
                        on_wait=[waits[-1]], on_update=list(si.on_update)
                    )
                    nops = [
                        mybir.InstNoOp(
                            name=f"{inst.name}__wsplit{k}",
                            engine=inst.engine,
                            bass_nofuse=True,
                            sync_info=mybir.SyncInfo(on_wait=[w], on_update=[]),
                        )
                        for k, w in enumerate(waits[:-1])
                    ]
                    insts[i:i] = nops
                    i += len(nops)
                i += 1
        return super()._lower_ordered_insts(ordered)

    def _drain_and_barrier(self, tick_clock, wait_clock):
        gc = tick_clock.global_clock
        n = len(gc)
        for i in range(n):
            if gc[i] > 0:
                vc = VectorClock([0] * n)
                vc.require_at_least(i, gc[i])
                d = self.nc.sync.drain()
                wait_clock.add_sem_waits(d.ins, ScopedClock({None: vc}))
        self.nc.all_engine_barrier()
        assert self.sems is not None
        popped = self.nc._tile_sem_poison_stack.pop()
        assert popped is self._sem_poison
        self.nc.clear_and_free_semaphores(list(self.sems.allocated().values()))
        self.nc.all_engine_barrier()


def _act_recip(eng, out, in_):
    imm = lambda v: mybir.ImmediateValue(dtype=f32, value=v)
    return eng.add_instruction(mybir.InstActivation(
        name=eng.bass.get_next_instruction_name(),
        func=mybir.ActivationFunctionType.Reciprocal,
        ins=[eng.lower_ap(in_), imm(0.0), imm(1.0), imm(0.0)],
        outs=[eng.lower_ap(out)],
    ))


def build_nc(rb_mode=RB_MODE, patt_bufs=3, pmm_bufs=3, ppv_bufs=2, e_bufs=10):
    # rb_mode: how exp(S+rb) is formed:
    #   "mul_pool"  E = exp(S) * erb on gpsimd
    #   "mul_dve"   E = exp(S) * erb on DVE
    #   "mul_split" alternate gpsimd/DVE by head parity
    #   "ident_pe"  S += rb via identity matmul on PE, E = exp(S)
    nc = bass.Bass("TRN2", target_bir_lowering=False, debug=False)
    Exp = mybir.ActivationFunctionType.Exp

    xT_d = nc.dram_tensor("xT", [BC, C, N], bf16, kind="ExternalInput").ap()
    qkw_d = nc.dram_tensor("qkw", [C, 2 * C], bf16, kind="ExternalInput").ap()
    vw_d = nc.dram_tensor("vw", [C, C], bf16, kind="ExternalInput").ap()
    pw_d = nc.dram_tensor("pw", [C, C], bf16, kind="ExternalInput").ap()
    pb_d = nc.dram_tensor("pb", [1, C], f32, kind="ExternalInput").ap()
    qb_d = nc.dram_tensor("qb", [128, 6], f32, kind="ExternalInput").ap()

    erb_d = nc.dram_tensor("erb", [H, N, NP], bf16, kind="ExternalInput").ap()
    out_d = nc.dram_tensor("out", [BC * N, C], f32, kind="ExternalOutput").ap()

    MT = ((0, 128), (128, 69))  # (row offset, rows) m-tiles of 197

    with TileContextFixed(nc) as tc, ExitStack() as ctx:
        consts = ctx.enter_context(tc.tile_pool(name="consts", bufs=1))
        xt_p = ctx.enter_context(tc.tile_pool(name="xt", bufs=2))
        qkt_p = ctx.enter_context(tc.tile_pool(name="qkt", bufs=3))
        v_p = ctx.enter_context(tc.tile_pool(name="v", bufs=2))
        at_p = ctx.enter_context(tc.tile_pool(name="at", bufs=3))
        e_p = ctx.enter_context(tc.tile_pool(name="e", bufs=e_bufs))
        rcp_p = ctx.enter_context(tc.tile_pool(name="rcp", bufs=4))
        stage_p = ctx.enter_context(tc.tile_pool(name="stage", bufs=3))
        pmm = ctx.enter_context(tc.tile_pool(name="pmm", bufs=pmm_bufs, space="PSUM"))
        patt = ctx.enter_context(tc.tile_pool(name="patt", bufs=patt_bufs, space="PSUM"))
        ppv = ctx.enter_context(tc.tile_pool(name="ppv", bufs=ppv_bufs, space="PSUM"))

        xt_pre = xt_p.tile([128, 6, NP], bf16)
        for j in range(BLK):
            nc.sync.dma_start(
                out=xt_pre[:, :, j * N:(j + 1) * N],
                in_=xT_d[j].rearrange("(k p) n -> p k n", p=128),
            )
        qkw_s = consts.tile([128, 6, 2 * C], bf16)
        qkw_r = qkw_d.rearrange("(k p) c -> p k c", p=128)
        for k in range(6):
            nc.sync.dma_start(out=qkw_s[:, k, :], in_=qkw_r[:, k, :])
        qb_s = consts.tile([128, 6], f32)
        nc.sync.dma_start(out=qb_s[:], in_=qb_d[:])
        vw_s = consts.tile([128, 6, C], bf16)
        pw_s = consts.tile([128, 6, C], bf16)
        erb0_s = consts.tile([128, H, NP], bf16)
        erb1_s = consts.tile([69, H, NP], bf16)
        pbb_s = consts.tile([128, C], f32)
        ones64 = consts.tile([128, 64], bf16)
        nc.gpsimd.memset(ones64[:], 1.0)

        SB = NB // 2                      # superblocks of 4 batches
        MT7 = [(g, min(128, 2 * NP - g)) for g in range(0, 2 * NP, 128)]

        for sb in range(SB):
            at_s = at_p.tile([128, 6, 2 * NP], bf16)
            for bh in range(2):
                blk = sb * 2 + bh
                b0 = blk * BLK
                off = bh * NP

                if blk == 0:
                    xt_s = xt_pre
                else:
                    xt_s = xt_p.tile([128, 6, NP], bf16)
                    for j in range(BLK):
                        nc.sync.dma_start(
                            out=xt_s[:, :, j * N:(j + 1) * N],
                            in_=xT_d[b0 + j].rearrange("(k p) n -> p k n", p=128),
                        )

                # ---- qkT [12 x 128, NP] bf16 (q part gets scaled bias) ----
                qkt_s = qkt_p.tile([128, H, NP], bf16)
                anchors = {}
                for mi in range(12):
                    ps = pmm.tile([128, NP], f32, tag="pmm")
                    for k in range(6):
                        mm = nc.tensor.matmul(
                            ps[:],
                            lhsT=qkw_s[:, k, mi * 128:(mi + 1) * 128],
                            rhs=xt_s[:, k, :],
                            start=(k == 0), stop=(k == 5),
                        )
                        if blk == 0 and mi in (0, 6) and k == 0:
                            anchors[mi] = mm.ins
                    if mi < 6:
                        nc.vector.tensor_scalar_add(qkt_s[:, mi, :], ps[:], qb_s[:, mi:mi + 1])
                    else:
                        nc.vector.tensor_copy(out=qkt_s[:, mi, :], in_=ps[:])

                if blk == 0:
                    d1 = nc.gpsimd.dma_start(out=vw_s[:], in_=vw_d.rearrange("(k p) c -> p k c", p=128))
                    d2 = nc.gpsimd.dma_start(out=erb0_s[:], in_=erb_d[:, 0:128, :].rearrange("h p n -> p h n"))
                    d3 = nc.gpsimd.dma_start(out=erb1_s[:], in_=erb_d[:, 128:197, :].rearrange("h p n -> p h n"))
                    d4 = nc.gpsimd.dma_start(out=pw_s[:], in_=pw_d.rearrange("(k p) c -> p k c", p=128))
                    d5 = nc.gpsimd.dma_start(out=pbb_s[:], in_=bass.AP(tensor=pb_d.tensor, offset=0,
                                                                       ap=[[0, 128], [1, C]]))
                    for d in (d1, d2, d3):
                        add_dep_helper(d.ins, anchors[0], reason="defer const load past startup")
                    for d in (d4, d5):
                        add_dep_helper(d.ins, anchors[6], reason="defer const load past startup")

                # ---- v natural [NP, 12 heads x 64] bf16 ----
                v_s = v_p.tile([128, BLK, 2, H, 64], bf16)
                for j in range(BLK):
                    for t, (r0, msz) in enumerate(MT):
                        for nt in range(2):
                            ps = pmm.tile([128, 384], f32, tag="pmm")
                            for k in range(6):
                                nc.tensor.matmul(
                                    ps[0:msz, :],
                                    lhsT=xt_s[:, k, j * N + r0: j * N + r0 + msz],
                                    rhs=vw_s[:, k, nt * 384:(nt + 1) * 384],
                                    start=(k == 0), stop=(k == 5),
                                )
                            nc.vector.tensor_copy(
                                out=v_s[0:msz, j, t, nt * 6:(nt + 1) * 6, :],
                                in_=ps[0:msz, :].rearrange("p (h d) -> p h d", h=6),
                            )

                # ---- attention: scores psum holds both batches of the half-block
                # as two CLOSED groups; one exp + one exp(rb)-multiply ----
                for hp in range(6):
                    es = {}
                    for t, (r0, msz) in enumerate(MT):
                        erb_t = erb0_s if t == 0 else erb1_s
                        # Interleave the two heads' K=64 scores matmuls so each
                        # adjacent PE instruction targets a disjoint row group
                        # (0:64 vs 64:128) and the sub-arrays overlap them.
                        ps_a = patt.tile([128, NP], f32, tag="patt")
                        ps_b = patt.tile([128, NP], f32, tag="patt")
                        pss = {0: ps_a, 1: ps_b}
                        for j in range(BLK):
                            for hi in range(2):
                                nc.tensor.matmul(
                                    pss[hi][0:msz, j * N:(j + 1) * N],
                                    lhsT=qkt_s[64 * hi:64 * (hi + 1), 6 + hp,
                                               j * N + r0: j * N + r0 + msz],
                                    rhs=qkt_s[64 * hi:64 * (hi + 1), hp, j * N:(j + 1) * N],
                                    start=True, stop=True, skip_group_check=True,
                                )
                        for hi in range(2):
                            h = 2 * hp + hi
                            e = e_p.tile([128, NP], bf16, tag="e")
                            nc.scalar.activation(out=e[0:msz, :], in_=pss[hi][0:msz, :], func=Exp)
                            eng = nc.gpsimd if hi == 0 else nc.vector
                            eng.tensor_mul(e[0:msz, :], e[0:msz, :], erb_t[0:msz, h, :])
                            es[(t, hi)] = e
                    for j in range(BLK):
                        ps_o = ppv.tile([128, 2 * N], f32, tag="ppv")
                        for hi in range(2):
                            h = 2 * hp + hi
                            for t, (r0, msz) in enumerate(MT):
                                nc.tensor.matmul(
                                    ps_o[hi * 64:(hi + 1) * 64, 0:N],
                                    lhsT=v_s[0:msz, j, t, h, :],
                                    rhs=es[(t, hi)][0:msz, j * N:(j + 1) * N],
                                    start=(t == 0), stop=(t == 1),
                                    skip_group_check=True,
                                )
                        for hi in range(2):
                            for t, (r0, msz) in enumerate(MT):
                                nc.tensor.matmul(
                                    ps_o[hi * 64:(hi + 1) * 64, N:2 * N],
                                    lhsT=ones64[0:msz, :],
                                    rhs=es[(t, hi)][0:msz, j * N:(j + 1) * N],
                                    start=(t == 0), stop=(t == 1),
                                    skip_group_check=True,
                                )
                        rcp = rcp_p.tile([128, N], f32, tag="rcp")
                        _act_recip(nc.scalar, rcp[:], ps_o[:, N:2 * N])
                        nc.vector.tensor_mul(
                            at_s[:, hp, off + j * N:off + (j + 1) * N],
                            ps_o[:, 0:N], rcp[:],
                        )

            # ---- projection over the whole superblock (flat rows), + bias ----
            for g0, msz in MT7:
                stage = stage_p.tile([128, C], f32)
                for nt in range(2):
                    ps = pmm.tile([128, 384], f32, tag="pmm")
                    for k in range(6):
                        nc.tensor.matmul(
                            ps[0:msz, :],
                            lhsT=at_s[:, k, g0:g0 + msz],
                            rhs=pw_s[:, k, nt * 384:(nt + 1) * 384],
                            start=(k == 0), stop=(k == 5),
                        )
                    nc.vector.scalar_tensor_tensor(
                        out=stage[0:msz, nt * 384:(nt + 1) * 384],
                        in0=ps[0:msz, :], scalar=1.0,
                        in1=pbb_s[0:msz, nt * 384:(nt + 1) * 384],
                        op0=mybir.AluOpType.mult, op1=mybir.AluOpType.add,
                    )
                nc.sync.dma_start(
                    out=out_d[sb * 2 * NP + g0: sb * 2 * NP + g0 + msz, :],
                    in_=stage[0:msz, :],
                )
    return nc


_NC = None


def _get_nc():
    global _NC
    if _NC is None:
        _NC = build_nc()
    return _NC


_EXEC = None


def _get_exec():
    """Build the sharded PJRT executable once and reuse it across calls
    (run_bass_via_pjrt re-traces jax.jit on every invocation)."""
    global _EXEC
    if _EXEC is not None:
        return _EXEC
    import jax
    import numpy as _np
    from jax.sharding import Mesh, PartitionSpec
    from jax.experimental.shard_map import shard_map
    import concourse.mybir as mybir_
    from concourse import bass2jax

    nc = _get_nc()
    bass2jax.install_neuronx_cc_hook()
    partition_name = nc.partition_id_tensor.name if nc.partition_id_tensor else None
    in_names, out_names, out_avals = [], [], []
    for alloc in nc.m.functions[0].allocations:
        if not isinstance(alloc, mybir_.MemoryLocationSet):
            continue
        name = alloc.memorylocations[0].name
        if alloc.kind == "ExternalInput":
            if name != partition_name:
                in_names.append(name)
        elif alloc.kind == "ExternalOutput":
            out_names.append(name)
            out_avals.append(jax.core.ShapedArray(
                tuple(alloc.tensor_shape), mybir_.dt.np(alloc.dtype)))
    all_names = list(in_names)
    if partition_name is not None:
        all_names = all_names + [partition_name]

    def _body(*args):
        operands = list(args)
        if partition_name is not None:
            operands.append(bass2jax.partition_id_tensor())
        outs = bass2jax._bass_exec_p.bind(
            *operands,
            out_avals=tuple(out_avals),
            in_names=tuple(all_names),
            out_names=tuple(out_names),
            lowering_input_output_aliases=(),
            sim_require_finite=True,
            sim_require_nnan=True,
            nc=nc,
        )
        return tuple(outs)

    devices = jax.devices()[:N_CORES]
    mesh = Mesh(_np.asarray(devices), ("core",))
    # xT is data-parallel (split on axis 0); every other input is replicated,
    # so it uploads once instead of 8x.
    in_specs = tuple(
        PartitionSpec("core") if name == "xT" else PartitionSpec()
        for name in in_names
    )
    out_specs = (PartitionSpec("core"),) * len(out_avals)
    sharded = jax.jit(
        shard_map(_body, mesh=mesh, in_specs=in_specs, out_specs=out_specs,
                  check_rep=False),
        keep_unused=True,
    )
    _EXEC = (sharded, in_names, out_names, out_avals)
    return _EXEC


def _prep_host(x, qkv_w, q_bias, v_bias, rel_pos_table, proj_w, proj_b, rel_index,
               rb_mode="mul_pool"):
    x = np.asarray(x, np.float32)
    qkv_w = np.asarray(qkv_w, np.float32)
    xT = np.ascontiguousarray(x.transpose(0, 2, 1)).astype(ml_dtypes.bfloat16)
    qk_wT = np.ascontiguousarray(qkv_w[:2 * C].T)              # [C, 2C]
    qk_wT[:, :C] *= SCALE
    qk_wT = qk_wT.astype(ml_dtypes.bfloat16)
    qb = (np.asarray(q_bias, np.float32) * SCALE).reshape(6, 128).T.copy()  # [128, 6]
    v_wT = np.ascontiguousarray(qkv_w[2 * C:].T).astype(ml_dtypes.bfloat16)
    proj_wT = np.ascontiguousarray(np.asarray(proj_w, np.float32).T).astype(ml_dtypes.bfloat16)
    pb_eff = (np.asarray(proj_b, np.float32)
              + np.asarray(proj_w, np.float32) @ np.asarray(v_bias, np.float32))
    rb = np.asarray(rel_pos_table, np.float32)[
        np.asarray(rel_index).reshape(-1)].reshape(N, N, H)    # [n, m, h]
    rbT = np.exp(rb.transpose(2, 1, 0))
    rbT = np.concatenate([rbT] * BLK, axis=2)
    erbT = rbT.astype(ml_dtypes.bfloat16)
    return xT, qk_wT, qb, v_wT, proj_wT, pb_eff.reshape(1, C), erbT


def kernel(x, qkv_w, q_bias, v_bias, rel_pos_table, proj_w, proj_b, rel_index):
    xT, qk_wT, qb, v_wT, proj_wT, pb_eff, erbT = _prep_host(
        x, qkv_w, q_bias, v_bias, rel_pos_table, proj_w, proj_b, rel_index,
        rb_mode=RB_MODE)
    per_core = {
        "xT": xT,                                   # [B, C, N] -> split on axis 0
        "qkw": qk_wT, "vw": v_wT, "pw": proj_wT,
        "pb": pb_eff, "qb": qb, "erb": erbT,
    }
    try:
        sharded, in_names, out_names, out_avals = _get_exec()
        concat_in = [np.ascontiguousarray(per_core[name]) for name in in_names]
        out_arrs = sharded(*concat_in)
        out = np.asarray(out_arrs[out_names.index("out")]).reshape(B, N, C)
    except Exception:
        # Robust fallback: the stock SPMD runner (slower per call, same NEFF).
        in_maps = []
        for c in range(N_CORES):
            m = {k: v for k, v in per_core.items() if k != "xT"}
            m["xT"] = np.ascontiguousarray(xT[c * BC:(c + 1) * BC])
            in_maps.append(m)
        res = run_bass_kernel_spmd(_get_nc(), in_maps, core_ids=list(range(N_CORES)))
        out = np.concatenate(
            [res.results[c]["out"].reshape(BC, N, C) for c in range(N_CORES)], axis=0)
    return out.astype(np.float32)
